# revision 6
# baseline (speedup 1.0000x reference)
"""AngleEnsemble TRN2 kernel: von Mises mean-shift via Jacobi-Anger moments.

Math: softmax mixture w = (1/3) sum_m softmax(logits_m). Mean-shift iterates
theta <- atan2(S(theta), C(theta)) with C,S = sum_n w_n exp(k cos(theta-theta_n)) {cos,sin}theta_n.
Expanding exp(kappa cos phi) = I0 + 2 sum_k Ik cos(k phi) (truncated at K), C and S
become trig polynomials in theta whose per-batch coefficients are linear in w:
one fp16 matmul exp(logits) @ F' [360, 1+2(2K+1)] produces [Z | coeffC | coeffS]
(Z = softmax normalizer via the ones column). Iterations then need only
cos/sin(k theta), generated from the unit vector z=(cos,sin) by complex doubling
on the VectorEngine, and atan2+cos+sin collapse into a Newton-rsqrt normalize —
no transcendentals, no ACT table switches.
"""
import numpy as np
from contextlib import ExitStack

import concourse.bass as bass
import concourse.bacc as bacc
import concourse.mybir as mybir
from concourse.tile import TileContext
from concourse.bass_utils import run_bass_kernel_spmd

F32 = mybir.dt.float32
F16 = mybir.dt.float16
I32 = mybir.dt.int32
AF = mybir.ActivationFunctionType
OP = mybir.AluOpType

M, B, N = 3, 65536, 360
NCORES = 8
BS = B // NCORES          # 8192 batch rows per core
KORD = 12                 # Jacobi-Anger truncation order
NPC = 2 * KORD + 1        # 25 power columns [c_0..c_K | s_1..s_K]
NCOL = 1 + 2 * NPC        # 51 = Z | coeffC | coeffS
NK = 120                  # n-chunk (3 chunks of 120 = 360)
SUP = 2048                # b superchunk for DMA/exp staging
NSUP = BS // SUP          # 4
NJ = BS // 128            # 64 column-groups of 128 b
MS_ITERS = 10
MAGIC = 0x5F3759DF


def _newton_rsqrt(nc, pool, r2, out, steps=2, tag="nr"):
    """out = 1/sqrt(r2), both [128, W] f32 SBUF tiles. Pure DVE."""
    shape = list(r2.shape)
    y = pool.tile(shape, F32, name=f"{tag}_y", tag=f"{tag}_y")
    t = pool.tile(shape, F32, name=f"{tag}_t", tag=f"{tag}_t")
    # seed via int bit trick: y = bits(MAGIC - (i >> 1))
    r2i = r2.bitcast(I32)
    yi = y.bitcast(I32)
    nc.vector.tensor_scalar(yi, r2i, 1, None, OP.logical_shift_right)
    nc.vector.tensor_scalar(yi, yi, -1, MAGIC, OP.mult, OP.add)
    for _ in range(steps):
        nc.vector.tensor_tensor(t[:], y[:], y[:], OP.mult)        # y*y
        nc.vector.tensor_tensor(t[:], t[:], r2[:], OP.mult)       # x*y*y
        nc.vector.tensor_scalar(t[:], t[:], -0.5, 1.5, OP.mult, OP.add)
        nc.vector.tensor_tensor(y[:], y[:], t[:], OP.mult)
    nc.vector.tensor_copy(out[:], y[:])
    return out


def build(nc: bass.Bass):
    lg = nc.declare_dram_parameter("logitsT", [M, N, BS], F32, isOutput=False)
    sv = nc.declare_dram_parameter("sin_vecT", [3, BS], F32, isOutput=False)
    fp = nc.declare_dram_parameter("Fp", [3, NK, NCOL], F16, isOutput=False)
    w1b = nc.declare_dram_parameter("W1b", [5, 128], F32, isOutput=False)
    w2 = nc.declare_dram_parameter("W2", [128, 2], F32, isOutput=False)
    b2r = nc.declare_dram_parameter("b2r", [128, 2], F32, isOutput=False)
    out = nc.declare_dram_parameter("out", [BS, 2], F32, isOutput=True)
    ctmp = nc.dram_tensor("ctmp", [128, NJ], F32)
    stmp = nc.dram_tensor("stmp", [128, NJ], F32)

    with TileContext(nc) as tc, ExitStack() as ctx:
        consts = ctx.enter_context(tc.tile_pool(name="consts", bufs=1))
        state = ctx.enter_context(tc.tile_pool(name="state", bufs=1))
        stage = ctx.enter_context(tc.tile_pool(name="stage", bufs=3))
        epool = ctx.enter_context(tc.tile_pool(name="epool", bufs=2))
        ph1 = ctx.enter_context(tc.tile_pool(name="ph1", bufs=3))
        ph2 = ctx.enter_context(tc.tile_pool(name="ph2", bufs=2))
        headp = ctx.enter_context(tc.tile_pool(name="headp", bufs=3))
        psum = ctx.enter_context(tc.tile_pool(name="psum", bufs=4, space="PSUM"))
        psumh = ctx.enter_context(tc.tile_pool(name="psumh", bufs=2, space="PSUM"))

        # ---- constants ----
        fp_t = consts.tile([NK, 3, NCOL], F16)
        for k in range(3):
            nc.sync.dma_start(out=fp_t[:, k, :], in_=fp[k])
        w1b_t = consts.tile([5, 128], F32)
        nc.sync.dma_start(out=w1b_t[:], in_=w1b[:, :])
        w2_t = consts.tile([128, 2], F32)
        nc.sync.dma_start(out=w2_t[:], in_=w2[:, :])
        b2r_t = consts.tile([128, 2], F32)
        nc.sync.dma_start(out=b2r_t[:], in_=b2r[:, :])

        # ---- persistent per-b coefficient stash: b = 128*j + p ----
        coeffC = state.tile([128, NJ, NPC], F32)
        coeffS = state.tile([128, NJ, NPC], F32)

        # ================= phase 1: moments =================
        for s in range(NSUP):
            b0 = s * SUP
            for m in range(M):
                es = []
                for k in range(3):
                    lg_t = stage.tile([NK, SUP], F32, name=f"lg_{s}_{m}_{k}", tag="lg")
                    nc.gpsimd.dma_start(
                        out=lg_t[:], in_=lg[m, k * NK:(k + 1) * NK, b0:b0 + SUP]
                    )
                    e_t = epool.tile([NK, SUP], F16, name=f"e_{s}_{m}_{k}", tag=f"e{k}")
                    nc.scalar.activation(out=e_t[:], in_=lg_t[:], func=AF.Exp)
                    es.append(e_t)
                for g in range(SUP // 512):  # groups of 4 x 128 b
                    ps = psum.tile([128, 4, NCOL], F32, name=f"mom_{s}_{m}_{g}", tag="mom")
                    for j in range(4):
                        cols = g * 512 + j * 128
                        for k in range(3):
                            nc.tensor.matmul(
                                ps[:, j, :],
                                es[k][:, cols:cols + 128],
                                fp_t[:, k, :],
                                start=(k == 0),
                                stop=(k == 2),
                            )
                    rz = ph1.tile([128, 4], F32, name=f"rz_{s}_{m}_{g}", tag="rz")
                    nc.vector.reciprocal(rz[:], ps[:, :, 0])
                    jc = s * (SUP // 128) + g * 4
                    bc = rz[:, :, None].broadcast_to([128, 4, NPC])
                    csl = coeffC[:, jc:jc + 4, :]
                    ssl = coeffS[:, jc:jc + 4, :]
                    if m == 0:
                        nc.vector.tensor_tensor(csl, ps[:, :, 1:1 + NPC], bc, OP.mult)
                        nc.vector.tensor_tensor(ssl, ps[:, :, 1 + NPC:NCOL], bc, OP.mult)
                    else:
                        tmc = ph1.tile([128, 4, NPC], F32, name=f"tmc_{s}_{m}_{g}", tag="tmc")
                        tms = ph1.tile([128, 4, NPC], F32, name=f"tms_{s}_{m}_{g}", tag="tms")
                        nc.vector.tensor_tensor(tmc[:], ps[:, :, 1:1 + NPC], bc, OP.mult)
                        nc.vector.tensor_tensor(csl, csl, tmc[:], OP.add)
                        nc.vector.tensor_tensor(tms[:], ps[:, :, 1 + NPC:NCOL], bc, OP.mult)
                        nc.vector.tensor_tensor(ssl, ssl, tms[:], OP.add)

        # ================= phase 2: mean-shift iterations =================
        # powers tile P: col k = cos(k th) for k=0..K, col K+k = sin(k th) k=1..K
        P = state.tile([128, NJ, NPC], F32)
        nc.vector.memset(P[:, :, 0], 1.0)
        # init z = normalize(A1, B1) = normalize(CA_0, SB_0)
        Cred = state.tile([128, NJ], F32)
        Sred = state.tile([128, NJ], F32)
        nc.vector.tensor_copy(Cred[:], coeffC[:, :, 0])
        nc.vector.tensor_copy(Sred[:], coeffS[:, :, 0])

        def normalize_into_P():
            r2 = ph2.tile([128, NJ], F32, name="r2", tag="r2")
            t2 = ph2.tile([128, NJ], F32, name="t2", tag="t2")
            nc.vector.tensor_tensor(r2[:], Cred[:], Cred[:], OP.mult)
            nc.vector.tensor_tensor(t2[:], Sred[:], Sred[:], OP.mult)
            nc.vector.tensor_tensor(r2[:], r2[:], t2[:], OP.add)
            y = ph2.tile([128, NJ], F32, name="yn", tag="yn")
            _newton_rsqrt(nc, ph2, r2, y, steps=2, tag="nrm")
            nc.vector.tensor_tensor(P[:, :, 1], Cred[:], y[:], OP.mult)
            nc.vector.tensor_tensor(P[:, :, 1 + KORD], Sred[:], y[:], OP.mult)

        normalize_into_P()

        prodC = state.tile([128, NJ, NPC], F32)
        prodS = state.tile([128, NJ, NPC], F32)
        CI, SI = 1, 1 + KORD  # base col of c_1 / s_1 in P

        for it in range(MS_ITERS):
            # ---- powers by complex doubling: have z^1..z^m, make z^{m+1}..z^{m+w}
            ta = ph2.tile([128, NJ, 4], F32, name=f"ta_{it}", tag="ta")
            tb = ph2.tile([128, NJ, 4], F32, name=f"tb_{it}", tag="tb")
            mlen = 1
            while mlen < KORD:
                w = min(mlen, KORD - mlen)
                cm = P[:, :, CI + mlen - 1:CI + mlen].broadcast_to([128, NJ, w])
                sm = P[:, :, SI + mlen - 1:SI + mlen].broadcast_to([128, NJ, w])
                cj = P[:, :, CI:CI + w]
                sj = P[:, :, SI:SI + w]
                nc.vector.tensor_tensor(ta[:, :, :w], cm, cj, OP.mult)
                nc.vector.tensor_tensor(tb[:, :, :w], sm, sj, OP.mult)
                nc.vector.tensor_tensor(
                    P[:, :, CI + mlen:CI + mlen + w], ta[:, :, :w], tb[:, :, :w], OP.subtract
                )
                nc.vector.tensor_tensor(ta[:, :, :w], sm, cj, OP.mult)
                nc.vector.tensor_tensor(tb[:, :, :w], cm, sj, OP.mult)
                nc.vector.tensor_tensor(
                    P[:, :, SI + mlen:SI + mlen + w], ta[:, :, :w], tb[:, :, :w], OP.add
                )
                mlen += w
            # ---- dots: C = sum_k coeffC*P, S = sum_k coeffS*P
            nc.vector.tensor_tensor(prodC[:], coeffC[:], P[:], OP.mult)
            nc.vector.tensor_reduce(Cred[:], prodC[:], mybir.AxisListType.X, OP.add)
            nc.vector.tensor_tensor(prodS[:], coeffS[:], P[:], OP.mult)
            nc.vector.tensor_reduce(Sred[:], prodS[:], mybir.AxisListType.X, OP.add)
            normalize_into_P()

        # ================= head MLP =================
        # bounce c,s (P cols 1, 1+K) through DRAM to reshape [128, NJ] -> [1, BS]
        nc.sync.dma_start(out=ctmp[:, :], in_=P[:, :, 1])
        nc.sync.dma_start(out=stmp[:, :], in_=P[:, :, 1 + KORD])
        fusedT = state.tile([5, BS], F32)
        nc.sync.dma_start(out=fusedT[0:3, :], in_=sv[:, :])
        nc.sync.dma_start(
            out=fusedT[3:4, :].rearrange("r (j p) -> r j p", p=128),
            in_=ctmp.rearrange("p j -> j p")[None, :, :],
        )
        nc.sync.dma_start(
            out=fusedT[4:5, :].rearrange("r (j p) -> r j p", p=128),
            in_=stmp.rearrange("p j -> j p")[None, :, :],
        )

        out_all = state.tile([128, NJ, 2], F32)
        for j in range(NJ):
            ps1 = psumh.tile([128, 128], F32, name=f"h_{j}", tag="h")
            nc.tensor.matmul(
                ps1[:], w1b_t[:], fusedT[:, j * 128:(j + 1) * 128],
                start=True, stop=True,
            )
            hT = headp.tile([128, 128], F32, name=f"hT_{j}", tag="hT")
            nc.scalar.activation(out=hT[:], in_=ps1[:], func=AF.Relu)
            ps2 = psumh.tile([128, 2], F32, name=f"o_{j}", tag="o")
            nc.tensor.matmul(ps2[:], hT[:], w2_t[:], start=True, stop=True)
            nc.vector.tensor_tensor(out_all[:, j, :], ps2[:], b2r_t[:], OP.add)

        # final row-normalize: out /= max(|out|, 1e-12)
        sq = ph2.tile([128, NJ, 2], F32, name="sq", tag="sq")
        nc.vector.tensor_tensor(sq[:], out_all[:], out_all[:], OP.mult)
        r2o = ph2.tile([128, NJ], F32, name="r2o", tag="r2o")
        nc.vector.tensor_tensor(r2o[:], sq[:, :, 0], sq[:, :, 1], OP.add)
        yo = ph2.tile([128, NJ], F32, name="yo", tag="yo")
        _newton_rsqrt(nc, ph2, r2o, yo, steps=3, tag="nro")
        nc.vector.tensor_scalar(yo[:], yo[:], 1e12, None, OP.min)
        nc.vector.tensor_tensor(
            out_all[:], out_all[:], yo[:, :, None].broadcast_to([128, NJ, 2]), OP.mult
        )
        nc.sync.dma_start(
            out=out.rearrange("(j p) c -> p j c", p=128), in_=out_all[:]
        )


def _build_Fp():
    """F' [3, NK, NCOL] fp16: exp-logits -> [Z | coeffC(25) | coeffS(25)]."""
    # I_k(10) for k=0..13, hardcoded (scipy.special.iv(k, 10.0))
    iv10 = [
        2815.716628466254, 2670.988303701255, 2281.518967726004,
        1758.380716166120, 1226.490565693291, 777.1882064830589,
        449.3022898718774, 238.0255847757819, 116.0661461102767,
        52.31922632375539, 21.89170616206518, 8.536924495442690,
        3.119276255343020, 1.071597692949700,
    ]
    K = KORD
    n = np.arange(N)
    th = 2 * np.pi * n / N
    c = np.array([iv10[0]] + [2 * iv10[k] for k in range(1, K + 2)])
    A = np.cos(np.outer(np.arange(K + 2), th))   # [K+2, N]
    Bm = np.sin(np.outer(np.arange(K + 2), th))
    cols = [np.ones(N)]
    cols.append(c[0] * A[1])                          # CA_0
    for k in range(1, K + 1):
        cols.append(c[k] / 2 * (A[k - 1] + A[k + 1]))  # CA_k
    for k in range(1, K + 1):
        cols.append(c[k] / 2 * (Bm[k - 1] + Bm[k + 1]))  # CB_k
    cols.append(c[0] * Bm[1])                         # SB_0
    for k in range(1, K + 1):
        cols.append(c[k] / 2 * (Bm[k + 1] - Bm[k - 1]))  # SB_k
    for k in range(1, K + 1):
        cols.append(c[k] / 2 * (A[k - 1] - A[k + 1]))  # SA_k
    Fp = np.stack(cols, axis=1).astype(np.float16)    # [N, NCOL]
    return np.ascontiguousarray(Fp.reshape(3, NK, NCOL))


_NC_CACHE = {}


def _get_nc():
    if "nc" not in _NC_CACHE:
        nc = bacc.Bacc("TRN2", target_bir_lowering=False, debug=False,
                       enable_asserts=True, num_devices=NCORES)
        build(nc)
        nc.compile()
        _NC_CACHE["nc"] = nc
    return _NC_CACHE["nc"]


def kernel(von_logits, sin_vec, W1, b1, W2, b2, _trace=False, _trace_kwargs=None):
    vT = np.ascontiguousarray(
        np.asarray(von_logits, np.float32).transpose(0, 2, 1)
    )  # [M, N, B]
    svT = np.concatenate([
        np.asarray(sin_vec, np.float32).T,
        np.ones((1, B), np.float32),
    ], axis=0)  # [3, B] rows: sv0, sv1, ones
    Fp = _build_Fp()
    W1f = np.asarray(W1, np.float32)
    W1b = np.ascontiguousarray(np.concatenate(
        [W1f[0:2], np.asarray(b1, np.float32)[None, :], W1f[2:4]], 0))
    W2f = np.ascontiguousarray(np.asarray(W2, np.float32))
    b2rep = np.ascontiguousarray(np.broadcast_to(np.asarray(b2, np.float32), (128, 2)))

    in_maps = []
    for ci in range(NCORES):
        sl = slice(ci * BS, (ci + 1) * BS)
        in_maps.append({
            "logitsT": np.ascontiguousarray(vT[:, :, sl]),
            "sin_vecT": np.ascontiguousarray(svT[:, sl]),
            "Fp": Fp, "W1b": W1b, "W2": W2f, "b2r": b2rep,
        })

    nc = _get_nc()
    kw = {}
    if _trace:
        kw = {"trace": True, "trace_kwargs": _trace_kwargs or {}}
    res = run_bass_kernel_spmd(nc, in_maps, core_ids=list(range(NCORES)), **kw)
    outs = [r["out"] for r in res.results]
    full = np.concatenate(outs, axis=0).astype(np.float32)
    if _trace:
        kernel._last_results = res
    return full
